# revision 77
# baseline (speedup 1.0000x reference)
"""Trainium2 Bass kernel for nn_MultiHeadAttention (LN -> QKV -> MHA -> FC -> +residual).

Sharding: data-parallel over batch (B=8 -> 1 batch element per NeuronCore).
Returns (out, attn) matching the jax reference.

Per-core pipeline (T=1024, C=1024, H=16, Dk=64):
  A. LN stats (bn_stats) + normalize in [T,C]; PE-transpose tiles -> h^T [C,T]
     with ln_g/ln_b affine fused into the PSUM evacuation (output f32r).
  B. q^T,k^T = (W_qkv as stationary).T @ h^T  -> [2048, T] f32r
     v       = h^T as stationary @ W_v        -> [T, 1024] natural, stored bf16
     with a ones column appended per head (flash denominator trick).
  C. per head: S  = q^T.T @ k^T   (natural)  -> exp via ACT with accum_out
                  -> row sums l; attn_out = P * (1/l)  (DMA to HBM)
               S^T = k^T.T @ q^T  (transposed) -> exp -> P^T bf16
               O'^T = v_aug.T @ P^T  (65 rows: 64 of O^T + row of l)
                  -> normalize O^T via gpsimd partition_broadcast of 1/l row
     QKV matmuls of the next pair and PV matmuls of the previous head are
     pumped as "filler" PE work into the ACT-paced exp phases.
  D. out = O^T.T @ W_fc + b_fc + x   (residual+bias fused into evacuation)

All matmuls use float32r (TF32-like, full PE rate, ~1.4e-4 rel err) except
PV/FC which are bf16. exp serves as the PSUM evacuation for both S layouts,
so the 16.8M-element attention matrix crosses ACT/DVE the minimum number of
times (exp x2, normalize x1).
"""
import sys

sys.path.insert(0, '/opt/trn_rl_repo')

import numpy as np

import concourse.bacc as bacc
import concourse.bass as bass
import concourse.mybir as mybir
import concourse.tile as tile
from concourse.bass_utils import run_bass_kernel_spmd
from concourse.masks import make_identity

F32 = mybir.dt.float32
F32R = mybir.dt.float32r
BF16 = mybir.dt.bfloat16
AF = mybir.ActivationFunctionType
OP = mybir.AluOpType

B, T, C = 8, 1024, 1024
H, DK = 16, 64
KO = C // 128          # 8 contraction chunks
TT = T // 128          # 8 token tiles
LN_EPS = 1e-5
SCALE = 1.0 / 8.0      # 1/sqrt(DK)
N_CORES = 8


def build_nc(trace_label=""):
    nc = bacc.Bacc("TRN2", target_bir_lowering=False, debug=False,
                   num_devices=N_CORES)
    x = nc.dram_tensor("x", [T, C], F32, kind="ExternalInput")
    ln_g = nc.dram_tensor("ln_g", [C], F32, kind="ExternalInput")
    ln_b = nc.dram_tensor("ln_b", [C], F32, kind="ExternalInput")
    w_qkv = nc.dram_tensor("w_qkv", [C, 3 * C], F32, kind="ExternalInput")
    b_qkv = nc.dram_tensor("b_qkv", [3 * C], F32, kind="ExternalInput")
    w_fc = nc.dram_tensor("w_fc", [C, C], F32, kind="ExternalInput")
    b_fc = nc.dram_tensor("b_fc", [C], F32, kind="ExternalInput")
    out = nc.dram_tensor("out", [T, C], F32, kind="ExternalOutput")
    attn = nc.dram_tensor("attn", [H, T, T], F32, kind="ExternalOutput")

    with tile.TileContext(nc) as tc:
        build_body(nc, tc, x, ln_g, ln_b, w_qkv, b_qkv, w_fc, b_fc, out, attn)
    nc.compile()
    return nc


def build_body(nc, tc, x, ln_g, ln_b, w_qkv, b_qkv, w_fc, b_fc, out, attn):
    from contextlib import ExitStack

    P = 128

    singles_cm = tc.tile_pool(name="singles", bufs=1)
    singles = singles_cm.__enter__()

    ident = singles.tile([P, P], F32)
    make_identity(nc, ident)

    eps_sb = singles.tile([P, 1], F32)
    nc.vector.memset(eps_sb, LN_EPS)

    # warm the Exp activation table during stage A (avoids a LoadActFuncSet
    # stall at the first attention exp)
    expwarm = singles.tile([P, 1], F32)
    nc.scalar.activation(expwarm, eps_sb, AF.Exp)

    # per-chunk ln scale/bias columns: g_sb[:, j] = ln_g[j*128:(j+1)*128]
    g_sb = singles.tile([P, KO], F32)
    b_sb = singles.tile([P, KO], F32)
    nc.sync.dma_start(g_sb, ln_g.rearrange("(o p) -> p o", p=P))
    nc.sync.dma_start(b_sb, ln_b.rearrange("(o p) -> p o", p=P))

    # qk bias columns (features 0..2047)
    bqk_sb = singles.tile([P, 16], F32)
    nc.sync.dma_start(bqk_sb, b_qkv[0:2 * C].rearrange("(o p) -> p o", p=P))

    # resident big tensors
    hT = singles.tile([P, KO, T], F32R)         # h^T
    v_sb = singles.tile([P, TT, H, DK + 1], BF16)  # v natural + ones column
    oT = singles.tile([P, KO, T], BF16)         # O^T (normalized)

    nc.gpsimd.memset(v_sb[:, :, :, DK:DK + 1], 1.0)

    # ------- Stage A: LN + transpose + v = h @ W_v (fused per t-tile) -------
    with ExitStack() as stA:
        pa = stA.enter_context(tc.tile_pool(name="pa", bufs=3))
        pax = stA.enter_context(tc.tile_pool(name="pax", bufs=1))
        pwv = stA.enter_context(tc.tile_pool(name="pwv", bufs=1))
        psA = stA.enter_context(tc.tile_pool(name="psA", bufs=3, space="PSUM"))
        psV = stA.enter_context(tc.tile_pool(name="psV", bufs=3, space="PSUM"))

        # v-bias broadcast to all 128 partitions via DMA replicate
        bv_bcast = pwv.tile([P, C], F32)
        bv_ap = bass.AP(tensor=b_qkv.ap().tensor, offset=2 * C, ap=[[0, P], [1, C]])
        nc.gpsimd.dma_start(out=bv_bcast, in_=bv_ap)
        wv_r = pwv.tile([P, KO, C], F32R)
        nc.gpsimd.dma_start(
            out=wv_r,
            in_=w_qkv.rearrange("(ko ki) f -> ki ko f", ki=P)[:, :, 2 * C:3 * C])

        x_sb = pax.tile([P, TT, C], F32)
        x_t = x.rearrange("(ti p) c -> p ti c", p=P)
        for i in range(TT):
            nc.sync.dma_start(x_sb[:, i], x_t[:, i])
            stats = pa.tile([P, 2, 6], F32, tag="stats")
            nc.vector.bn_stats(out=stats[:, 0, :], in_=x_sb[:, i, 0:512])
            nc.vector.bn_stats(out=stats[:, 1, :], in_=x_sb[:, i, 512:1024])
            mv = pa.tile([P, 2], F32, tag="mv")
            nc.vector.bn_aggr(out=mv, in_=stats)
            std = pa.tile([P, 1], F32, tag="std")
            nc.scalar.activation(std, mv[:, 1:2], AF.Sqrt, bias=eps_sb)
            rstd = pa.tile([P, 1], F32, tag="rstd")
            nc.vector.reciprocal(rstd, std)
            xhat = pa.tile([P, C], F32, tag="xhat")
            nc.vector.tensor_scalar(xhat, x_sb[:, i], mv[:, 0:1], rstd,
                                    OP.subtract, OP.mult)
            for j in range(KO):
                ps_t = psA.tile([P, P], F32, tag="tr")
                nc.tensor.transpose(ps_t, xhat[:, j * P:(j + 1) * P], ident)
                # hT[j][:, i-range] = ps_t * g[j] + b[j]  (rounds to f32r)
                nc.vector.tensor_scalar(hT[:, j, i * P:(i + 1) * P], ps_t,
                                        g_sb[:, j:j + 1], b_sb[:, j:j + 1],
                                        OP.mult, OP.add)
            # v rows for this t-tile (uses hT[:, :, i-slice] just written)
            for half in range(2):
                ps_v = psV.tile([P, 512], F32, tag="v")
                for j in range(KO):
                    nc.tensor.matmul(ps_v, hT[:, j, i * P:(i + 1) * P],
                                     wv_r[:, j, half * 512:(half + 1) * 512],
                                     start=(j == 0), stop=(j == KO - 1))
                # bias-add + scatter into v_sb[:, i, h0:h0+8, 0:64] (bf16) in one op
                h0 = half * 8
                nc.vector.tensor_tensor(
                    v_sb[:, i, h0:h0 + 8, 0:DK],
                    ps_v.rearrange("p (h d) -> p h d", h=8),
                    bv_bcast[:, half * 512:(half + 1) * 512].rearrange(
                        "p (h d) -> p h d", h=8),
                    OP.add)

    # ---------------- Stage B+C: QKV interleaved with attention ----------------
    from collections import deque

    with ExitStack() as stC:
        pw = stC.enter_context(tc.tile_pool(name="pw", bufs=2))
        pqk = stC.enter_context(tc.tile_pool(name="pqk", bufs=2))
        pc = stC.enter_context(tc.tile_pool(name="pc", bufs=2))
        ppt = stC.enter_context(tc.tile_pool(name="ppt", bufs=3))
        pwf = stC.enter_context(tc.tile_pool(name="pwf", bufs=1))
        psS = stC.enter_context(tc.tile_pool(name="psS", bufs=2, space="PSUM"))
        psQ = stC.enter_context(tc.tile_pool(name="psQ", bufs=2, space="PSUM"))

        wf_r = pwf.tile([P, KO, C], BF16)
        nc.gpsimd.dma_start(out=wf_r,
                            in_=w_fc.rearrange("(ko ki) f -> ki ko f", ki=P))

        w_qkv_t = w_qkv.rearrange("(ko ki) f -> ki ko f", ki=P)

        # PE filler queue: chunks of PE-heavy work (next pair's QKV matmuls,
        # previous head's PV matmuls) pumped into the ACT-paced exp phases so
        # the PE never sits idle while ACT runs and vice versa.
        filler = deque()

        def pump():
            if filler:
                filler.popleft()()

        def qk_closures(p, qkt):
            """Closures computing q^T (m-tile p) and k^T (m-tile 8+p)."""
            wq_r = pw.tile([P, KO, 2 * P], F32R, tag="wqr", name=f"wq_{p}")
            nc.gpsimd.dma_start(out=wq_r[:, :, 0:P],
                                in_=w_qkv_t[:, :, p * P:(p + 1) * P])
            nc.gpsimd.dma_start(out=wq_r[:, :, P:2 * P],
                                in_=w_qkv_t[:, :, (8 + p) * P:(9 + p) * P])
            outs = []
            for mi, m in ((0, p), (1, 8 + p)):
                for half in range(2):
                    state = {}

                    def chunk(j0, mi=mi, m=m, half=half, state=state):
                        if j0 == 0:
                            state["ps"] = psQ.tile([P, 512], F32, tag="qo",
                                                   name="ps_qk")
                        ps_qk = state["ps"]
                        for j in range(j0, j0 + 2):
                            nc.tensor.matmul(
                                ps_qk, wq_r[:, j, mi * P:(mi + 1) * P],
                                hT[:, j, half * 512:(half + 1) * 512],
                                start=(j == 0), stop=(j == KO - 1))
                        if j0 + 2 == KO:
                            nc.vector.tensor_scalar(
                                qkt[:, mi, half * 512:(half + 1) * 512],
                                ps_qk, bqk_sb[:, m:m + 1], None, OP.add)
                    for j0 in range(0, KO, 2):
                        outs.append(lambda j0=j0, chunk=chunk: chunk(j0))
            return outs

        def pv_closures(h, PT):
            """P^T @ v -> O'^T (65 rows: O^T + l row), normalize into oT.
            ps_o is evacuated to SBUF immediately (frees the PSUM bank); the
            normalize chain (recip -> partition-broadcast -> mult) then runs
            on DVE + GpSimd only, keeping PE free."""
            qp = 64 * (h % 2)
            outs = []
            for n in range(2):
                state = {}

                def chunk(i0, n=n, state=state):
                    if i0 == 0:
                        state["ps"] = psQ.tile([P, 512], F32, tag="qo",
                                               name="ps_o")
                    ps_o = state["ps"]
                    for i in range(i0, i0 + 2):
                        nc.tensor.matmul(ps_o[0:65],
                                         v_sb[:, i, h, :],
                                         PT[:, i, n * 512:(n + 1) * 512],
                                         start=(i == 0), stop=(i == TT - 1))
                    if i0 + 2 == TT:
                        o_sb = pc.tile([65, 512], F32, tag="osb")
                        nc.vector.tensor_copy(o_sb, ps_o[0:65])
                        nc.vector.reciprocal(o_sb[64:65, :], o_sb[64:65, :])
                        rrow0 = pc.tile([1, 512], F32, tag="rrow0")
                        nc.gpsimd.tensor_copy(rrow0, o_sb[64:65, :])
                        rb = pc.tile([64, 512], F32, tag="rb")
                        nc.gpsimd.partition_broadcast(rb, rrow0)
                        nc.vector.tensor_tensor(
                            oT[qp:qp + 64, h // 2, n * 512:(n + 1) * 512],
                            o_sb[0:64], rb, OP.mult)
                for i0 in range(0, TT, 2):
                    outs.append(lambda i0=i0, chunk=chunk: chunk(i0))
            return outs

        # prologue: pair 0's q^T/k^T computed up front
        qkt_cur = pqk.tile([P, 2, T], F32R, tag="qkt", name="qkt_0")
        for c in qk_closures(0, qkt_cur):
            c()

        prev_pv = None            # (h, PT) whose PV is pending
        for p in range(8):        # head pair p -> heads 2p, 2p+1
            if p < 7:
                qkt_next = pqk.tile([P, 2, T], F32R, tag="qkt", name=f"qkt_{p+1}")
                filler.extend(qk_closures(p + 1, qkt_next))
            for h in (2 * p, 2 * p + 1):
                hp = 64 * (h % 2)
                # --- transposed S -> P^T (bf16) ---
                # exp ops merged to 1536 cols (3 PSUM banks) to amortize the
                # per-op ACT overhead; segments are elementwise-independent
                PT = ppt.tile([P, TT, T], BF16, tag="pt", name=f"PT_{h}")
                PTf = PT.rearrange("p a b -> p (a b)")
                col = 0
                for w in (1536, 1536, 1536, 1536, 1536, 512):
                    ps_st = psS.tile([P, 1536], F32, tag="s", name="ps_st")
                    for off in range(0, w, 512):
                        gcol = col + off
                        i, half = gcol // T, (gcol % T) // 512
                        nc.tensor.matmul(
                            ps_st[:, off:off + 512],
                            qkt_cur[hp:hp + 64, 1, i * P:(i + 1) * P],
                            qkt_cur[hp:hp + 64, 0, half * 512:(half + 1) * 512],
                            start=True, stop=True)
                    nc.scalar.activation(PTf[:, col:col + w], ps_st[:, :w],
                                         AF.Exp, scale=SCALE)
                    col += w
                    pump()
                    pump()

                # --- natural S -> attn output ---
                for i in range(TT):
                    ps_s = psS.tile([P, T], F32, tag="s", name="ps_s")
                    for half in range(2):
                        nc.tensor.matmul(
                            ps_s[:, half * 512:(half + 1) * 512],
                            qkt_cur[hp:hp + 64, 0, i * P:(i + 1) * P],
                            qkt_cur[hp:hp + 64, 1, half * 512:(half + 1) * 512],
                            start=True, stop=True)
                    pnat = pc.tile([P, T], F32, tag="pnat", bufs=4)
                    lcol = pc.tile([P, 1], F32, tag="lcol")
                    nc.scalar.activation(pnat, ps_s, AF.Exp, scale=SCALE,
                                         accum_out=lcol)
                    rcol = pc.tile([P, 1], F32, tag="rcol")
                    nc.vector.reciprocal(rcol, lcol)
                    asb = pc.tile([P, T], F32, tag="attn", bufs=3)
                    nc.vector.tensor_scalar(asb, pnat, rcol, None, OP.mult)
                    nc.sync.dma_start(attn[h, i * P:(i + 1) * P, :], asb)
                    pump()

                # queue PV of the previous head; this head's PV queues next
                if prev_pv is not None:
                    filler.extend(pv_closures(*prev_pv))
                prev_pv = (h, PT)
            qkt_cur = qkt_next

        for c in pv_closures(*prev_pv):
            c()
        while filler:
            pump()

    # ---------------- Stage D: FC + residual ----------------
    with ExitStack() as stD:
        pd = stD.enter_context(tc.tile_pool(name="pd", bufs=2))
        pdb = stD.enter_context(tc.tile_pool(name="pdb", bufs=1))
        psD = stD.enter_context(tc.tile_pool(name="psD", bufs=3, space="PSUM"))

        bfc_bcast = pdb.tile([P, C], F32)
        bfc_ap = bass.AP(tensor=b_fc.ap().tensor, offset=0, ap=[[0, P], [1, C]])
        nc.gpsimd.dma_start(out=bfc_bcast, in_=bfc_ap)

        for i in range(TT):
            x_re = pd.tile([P, C], F32, tag="xre")
            nc.sync.dma_start(x_re, x[i * P:(i + 1) * P, :])
            # fold b_fc into the residual in place
            nc.vector.tensor_tensor(x_re, x_re, bfc_bcast, OP.add)
            osb = pd.tile([P, C], F32, tag="osb")
            for half in range(2):
                ps_fc = psD.tile([P, 512], F32, tag="fc")
                for j in range(KO):
                    nc.tensor.matmul(ps_fc, oT[:, j, i * P:(i + 1) * P],
                                     wf_r[:, j, half * 512:(half + 1) * 512],
                                     start=(j == 0), stop=(j == KO - 1))
                nc.vector.tensor_tensor(osb[:, half * 512:(half + 1) * 512],
                                        ps_fc,
                                        x_re[:, half * 512:(half + 1) * 512],
                                        OP.add)
            nc.sync.dma_start(out[i * P:(i + 1) * P, :], osb)

    singles_cm.__exit__(None, None, None)


_NC_CACHE = None


def kernel(x, ln_g, ln_b, W_qkv, b_qkv, W_fc, b_fc):
    global _NC_CACHE
    if _NC_CACHE is None:
        _NC_CACHE = build_nc()
    nc = _NC_CACHE

    x = np.ascontiguousarray(np.asarray(x, dtype=np.float32))
    shared = {
        "ln_g": np.ascontiguousarray(np.asarray(ln_g, np.float32)),
        "ln_b": np.ascontiguousarray(np.asarray(ln_b, np.float32)),
        "w_qkv": np.ascontiguousarray(np.asarray(W_qkv, np.float32)),
        "b_qkv": np.ascontiguousarray(np.asarray(b_qkv, np.float32)),
        "w_fc": np.ascontiguousarray(np.asarray(W_fc, np.float32)),
        "b_fc": np.ascontiguousarray(np.asarray(b_fc, np.float32)),
    }
    in_maps = [{"x": x[b], **shared} for b in range(B)]
    res = run_bass_kernel_spmd(nc, in_maps, core_ids=list(range(N_CORES)))
    out = np.stack([res.results[b]["out"] for b in range(B)])
    attn = np.stack([res.results[b]["attn"] for b in range(B)])
    return out, attn


# revision 80
# speedup vs baseline: 1.0013x; 1.0013x over previous
"""Trainium2 Bass kernel for nn_MultiHeadAttention (LN -> QKV -> MHA -> FC -> +residual).

Sharding: data-parallel over batch (B=8 -> 1 batch element per NeuronCore).
Returns (out, attn) matching the jax reference.

Per-core pipeline (T=1024, C=1024, H=16, Dk=64):
  A. LN stats (bn_stats) + normalize in [T,C]; PE-transpose tiles -> h^T [C,T]
     with ln_g/ln_b affine fused into the PSUM evacuation (output f32r).
  B. q^T,k^T = (W_qkv as stationary).T @ h^T  -> [2048, T] f32r
     v       = h^T as stationary @ W_v        -> [T, 1024] natural, stored bf16
     with a ones column appended per head (flash denominator trick).
  C. per head: S  = q^T.T @ k^T   (natural)  -> exp via ACT with accum_out
                  -> row sums l; attn_out = P * (1/l)  (DMA to HBM)
               S^T = k^T.T @ q^T  (transposed) -> exp -> P^T bf16
               O'^T = v_aug.T @ P^T  (65 rows: 64 of O^T + row of l)
                  -> normalize O^T via gpsimd partition_broadcast of 1/l row
     QKV matmuls of the next pair and PV matmuls of the previous head are
     pumped as "filler" PE work into the ACT-paced exp phases.
  D. out = O^T.T @ W_fc + b_fc + x   (residual+bias fused into evacuation)

All matmuls use float32r (TF32-like, full PE rate, ~1.4e-4 rel err) except
PV/FC which are bf16. exp serves as the PSUM evacuation for both S layouts,
so the 16.8M-element attention matrix crosses ACT/DVE the minimum number of
times (exp x2, normalize x1).
"""
import sys

sys.path.insert(0, '/opt/trn_rl_repo')

import numpy as np

import concourse.bacc as bacc
import concourse.bass as bass
import concourse.mybir as mybir
import concourse.tile as tile
from concourse.bass_utils import run_bass_kernel_spmd
from concourse.masks import make_identity

F32 = mybir.dt.float32
F32R = mybir.dt.float32r
BF16 = mybir.dt.bfloat16
AF = mybir.ActivationFunctionType
OP = mybir.AluOpType

B, T, C = 8, 1024, 1024
H, DK = 16, 64
KO = C // 128          # 8 contraction chunks
TT = T // 128          # 8 token tiles
LN_EPS = 1e-5
SCALE = 1.0 / 8.0      # 1/sqrt(DK)
N_CORES = 8


def build_nc(trace_label=""):
    nc = bacc.Bacc("TRN2", target_bir_lowering=False, debug=False,
                   num_devices=N_CORES)
    x = nc.dram_tensor("x", [T, C], F32, kind="ExternalInput")
    ln_g = nc.dram_tensor("ln_g", [C], F32, kind="ExternalInput")
    ln_b = nc.dram_tensor("ln_b", [C], F32, kind="ExternalInput")
    w_qkv = nc.dram_tensor("w_qkv", [C, 3 * C], F32, kind="ExternalInput")
    b_qkv = nc.dram_tensor("b_qkv", [3 * C], F32, kind="ExternalInput")
    w_fc = nc.dram_tensor("w_fc", [C, C], F32, kind="ExternalInput")
    b_fc = nc.dram_tensor("b_fc", [C], F32, kind="ExternalInput")
    out = nc.dram_tensor("out", [T, C], F32, kind="ExternalOutput")
    attn = nc.dram_tensor("attn", [H, T, T], F32, kind="ExternalOutput")

    with tile.TileContext(nc) as tc:
        build_body(nc, tc, x, ln_g, ln_b, w_qkv, b_qkv, w_fc, b_fc, out, attn)
    nc.compile()
    return nc


def build_body(nc, tc, x, ln_g, ln_b, w_qkv, b_qkv, w_fc, b_fc, out, attn):
    from contextlib import ExitStack

    P = 128

    singles_cm = tc.tile_pool(name="singles", bufs=1)
    singles = singles_cm.__enter__()

    ident = singles.tile([P, P], F32)
    make_identity(nc, ident)

    eps_sb = singles.tile([P, 1], F32)
    nc.vector.memset(eps_sb, LN_EPS)

    # warm the Exp activation table during stage A (avoids a LoadActFuncSet
    # stall at the first attention exp)
    expwarm = singles.tile([P, 1], F32)
    nc.scalar.activation(expwarm, eps_sb, AF.Exp)

    # per-chunk ln scale/bias columns: g_sb[:, j] = ln_g[j*128:(j+1)*128]
    g_sb = singles.tile([P, KO], F32)
    b_sb = singles.tile([P, KO], F32)
    nc.sync.dma_start(g_sb, ln_g.rearrange("(o p) -> p o", p=P))
    nc.sync.dma_start(b_sb, ln_b.rearrange("(o p) -> p o", p=P))

    # qk bias columns (features 0..2047)
    bqk_sb = singles.tile([P, 16], F32)
    nc.sync.dma_start(bqk_sb, b_qkv[0:2 * C].rearrange("(o p) -> p o", p=P))

    # resident big tensors
    hT = singles.tile([P, KO, T], F32R)         # h^T
    v_sb = singles.tile([P, TT, H, DK + 1], BF16)  # v natural + ones column
    oT = singles.tile([P, KO, T], BF16)         # O^T (normalized)

    nc.gpsimd.memset(v_sb[:, :, :, DK:DK + 1], 1.0)

    # ------- Stage A: LN + transpose + v = h @ W_v (fused per t-tile) -------
    with ExitStack() as stA:
        pa = stA.enter_context(tc.tile_pool(name="pa", bufs=3))
        pax = stA.enter_context(tc.tile_pool(name="pax", bufs=1))
        pwv = stA.enter_context(tc.tile_pool(name="pwv", bufs=1))
        psA = stA.enter_context(tc.tile_pool(name="psA", bufs=3, space="PSUM"))
        psV = stA.enter_context(tc.tile_pool(name="psV", bufs=3, space="PSUM"))

        # v-bias broadcast to all 128 partitions via DMA replicate
        bv_bcast = pwv.tile([P, C], F32)
        bv_ap = bass.AP(tensor=b_qkv.ap().tensor, offset=2 * C, ap=[[0, P], [1, C]])
        nc.gpsimd.dma_start(out=bv_bcast, in_=bv_ap)
        wv_r = pwv.tile([P, KO, C], F32R)
        nc.gpsimd.dma_start(
            out=wv_r,
            in_=w_qkv.rearrange("(ko ki) f -> ki ko f", ki=P)[:, :, 2 * C:3 * C])

        x_sb = pax.tile([P, TT, C], F32)
        x_t = x.rearrange("(ti p) c -> p ti c", p=P)
        for i in range(TT):
            nc.sync.dma_start(x_sb[:, i], x_t[:, i])
            stats = pa.tile([P, 2, 6], F32, tag="stats")
            nc.vector.bn_stats(out=stats[:, 0, :], in_=x_sb[:, i, 0:512])
            nc.vector.bn_stats(out=stats[:, 1, :], in_=x_sb[:, i, 512:1024])
            mv = pa.tile([P, 2], F32, tag="mv")
            nc.vector.bn_aggr(out=mv, in_=stats)
            std = pa.tile([P, 1], F32, tag="std")
            nc.scalar.activation(std, mv[:, 1:2], AF.Sqrt, bias=eps_sb)
            rstd = pa.tile([P, 1], F32, tag="rstd")
            nc.vector.reciprocal(rstd, std)
            xhat = pa.tile([P, C], F32, tag="xhat")
            nc.vector.tensor_scalar(xhat, x_sb[:, i], mv[:, 0:1], rstd,
                                    OP.subtract, OP.mult)
            for j in range(KO):
                ps_t = psA.tile([P, P], F32, tag="tr")
                nc.tensor.transpose(ps_t, xhat[:, j * P:(j + 1) * P], ident)
                # hT[j][:, i-range] = ps_t * g[j] + b[j]  (rounds to f32r)
                nc.vector.tensor_scalar(hT[:, j, i * P:(i + 1) * P], ps_t,
                                        g_sb[:, j:j + 1], b_sb[:, j:j + 1],
                                        OP.mult, OP.add)
            # v rows for this t-tile (uses hT[:, :, i-slice] just written)
            for half in range(2):
                ps_v = psV.tile([P, 512], F32, tag="v")
                for j in range(KO):
                    nc.tensor.matmul(ps_v, hT[:, j, i * P:(i + 1) * P],
                                     wv_r[:, j, half * 512:(half + 1) * 512],
                                     start=(j == 0), stop=(j == KO - 1))
                # bias-add + scatter into v_sb[:, i, h0:h0+8, 0:64] (bf16) in one op
                h0 = half * 8
                nc.vector.tensor_tensor(
                    v_sb[:, i, h0:h0 + 8, 0:DK],
                    ps_v.rearrange("p (h d) -> p h d", h=8),
                    bv_bcast[:, half * 512:(half + 1) * 512].rearrange(
                        "p (h d) -> p h d", h=8),
                    OP.add)

    # ---------------- Stage B+C: QKV interleaved with attention ----------------
    from collections import deque

    with ExitStack() as stC:
        pw = stC.enter_context(tc.tile_pool(name="pw", bufs=2))
        pqk = stC.enter_context(tc.tile_pool(name="pqk", bufs=2))
        pc = stC.enter_context(tc.tile_pool(name="pc", bufs=2))
        ppt = stC.enter_context(tc.tile_pool(name="ppt", bufs=3))
        pwf = stC.enter_context(tc.tile_pool(name="pwf", bufs=1))
        psS = stC.enter_context(tc.tile_pool(name="psS", bufs=2, space="PSUM"))
        psQ = stC.enter_context(tc.tile_pool(name="psQ", bufs=2, space="PSUM"))

        wf_r = pwf.tile([P, KO, C], BF16)
        nc.gpsimd.dma_start(out=wf_r,
                            in_=w_fc.rearrange("(ko ki) f -> ki ko f", ki=P))

        w_qkv_t = w_qkv.rearrange("(ko ki) f -> ki ko f", ki=P)

        # PE filler queue: chunks of PE-heavy work (next pair's QKV matmuls,
        # previous head's PV matmuls) pumped into the ACT-paced exp phases so
        # the PE never sits idle while ACT runs and vice versa.
        filler = deque()

        def pump():
            if filler:
                filler.popleft()()

        def qk_closures(p, qkt):
            """Closures computing q^T (m-tile p) and k^T (m-tile 8+p)."""
            wq_r = pw.tile([P, KO, 2 * P], F32R, tag="wqr", name=f"wq_{p}")
            nc.gpsimd.dma_start(out=wq_r[:, :, 0:P],
                                in_=w_qkv_t[:, :, p * P:(p + 1) * P])
            nc.gpsimd.dma_start(out=wq_r[:, :, P:2 * P],
                                in_=w_qkv_t[:, :, (8 + p) * P:(9 + p) * P])
            outs = []
            for mi, m in ((0, p), (1, 8 + p)):
                for half in range(2):
                    state = {}

                    def chunk(j0, mi=mi, m=m, half=half, state=state):
                        if j0 == 0:
                            state["ps"] = psQ.tile([P, 512], F32, tag="qo",
                                                   name="ps_qk")
                        ps_qk = state["ps"]
                        for j in range(j0, j0 + 2):
                            nc.tensor.matmul(
                                ps_qk, wq_r[:, j, mi * P:(mi + 1) * P],
                                hT[:, j, half * 512:(half + 1) * 512],
                                start=(j == 0), stop=(j == KO - 1))
                        if j0 + 2 == KO:
                            nc.vector.tensor_scalar(
                                qkt[:, mi, half * 512:(half + 1) * 512],
                                ps_qk, bqk_sb[:, m:m + 1], None, OP.add)
                    for j0 in range(0, KO, 2):
                        outs.append(lambda j0=j0, chunk=chunk: chunk(j0))
            return outs

        def pv_closures(h, PT):
            """P^T @ v -> O'^T (65 rows: O^T + l row), normalize into oT.
            ps_o is evacuated to SBUF immediately (frees the PSUM bank); the
            normalize chain (recip -> partition-broadcast -> mult) then runs
            on DVE + GpSimd only, keeping PE free."""
            qp = 64 * (h % 2)
            outs = []
            for n in range(2):
                state = {}

                def chunk(i0, n=n, state=state):
                    if i0 == 0:
                        state["ps"] = psQ.tile([P, 512], F32, tag="qo",
                                               name="ps_o")
                    ps_o = state["ps"]
                    for i in range(i0, i0 + 2):
                        nc.tensor.matmul(ps_o[0:65],
                                         v_sb[:, i, h, :],
                                         PT[:, i, n * 512:(n + 1) * 512],
                                         start=(i == 0), stop=(i == TT - 1))
                    if i0 + 2 == TT:
                        o_sb = pc.tile([65, 512], F32, tag="osb")
                        nc.vector.tensor_copy(o_sb, ps_o[0:65])
                        nc.vector.reciprocal(o_sb[64:65, :], o_sb[64:65, :])
                        rrow0 = pc.tile([1, 512], F32, tag="rrow0")
                        nc.gpsimd.tensor_copy(rrow0, o_sb[64:65, :])
                        rb = pc.tile([64, 512], F32, tag="rb")
                        nc.gpsimd.partition_broadcast(rb, rrow0)
                        nc.vector.tensor_tensor(
                            oT[qp:qp + 64, h // 2, n * 512:(n + 1) * 512],
                            o_sb[0:64], rb, OP.mult)
                for i0 in range(0, TT, 2):
                    outs.append(lambda i0=i0, chunk=chunk: chunk(i0))
            return outs

        # prologue: pair 0's q^T/k^T computed up front
        qkt_cur = pqk.tile([P, 2, T], F32R, tag="qkt", name="qkt_0")
        for c in qk_closures(0, qkt_cur):
            c()

        prev_pv = None            # (h, PT) whose PV is pending
        for p in range(8):        # head pair p -> heads 2p, 2p+1
            if p < 7:
                qkt_next = pqk.tile([P, 2, T], F32R, tag="qkt", name=f"qkt_{p+1}")
                filler.extend(qk_closures(p + 1, qkt_next))
            for h in (2 * p, 2 * p + 1):
                hp = 64 * (h % 2)
                # --- transposed S -> P^T (bf16) ---
                # exp ops merged to 1536 cols (3 PSUM banks) to amortize the
                # per-op ACT overhead; segments are elementwise-independent
                PT = ppt.tile([P, TT, T], BF16, tag="pt", name=f"PT_{h}")
                PTf = PT.rearrange("p a b -> p (a b)")
                col = 0
                for w in (1536, 1536, 1536, 1536, 1536, 512):
                    ps_st = psS.tile([P, 1536], F32, tag="s", name="ps_st")
                    for off in range(0, w, 512):
                        gcol = col + off
                        i, half = gcol // T, (gcol % T) // 512
                        nc.tensor.matmul(
                            ps_st[:, off:off + 512],
                            qkt_cur[hp:hp + 64, 1, i * P:(i + 1) * P],
                            qkt_cur[hp:hp + 64, 0, half * 512:(half + 1) * 512],
                            start=True, stop=True)
                    nc.scalar.activation(PTf[:, col:col + w], ps_st[:, :w],
                                         AF.Exp, scale=SCALE)
                    col += w
                    pump()
                    pump()
                    pump()

                # --- natural S -> attn output ---
                for i in range(TT):
                    ps_s = psS.tile([P, T], F32, tag="s", name="ps_s")
                    for half in range(2):
                        nc.tensor.matmul(
                            ps_s[:, half * 512:(half + 1) * 512],
                            qkt_cur[hp:hp + 64, 0, i * P:(i + 1) * P],
                            qkt_cur[hp:hp + 64, 1, half * 512:(half + 1) * 512],
                            start=True, stop=True)
                    pnat = pc.tile([P, T], F32, tag="pnat", bufs=4)
                    lcol = pc.tile([P, 1], F32, tag="lcol")
                    nc.scalar.activation(pnat, ps_s, AF.Exp, scale=SCALE,
                                         accum_out=lcol)
                    rcol = pc.tile([P, 1], F32, tag="rcol")
                    nc.vector.reciprocal(rcol, lcol)
                    asb = pc.tile([P, T], F32, tag="attn", bufs=3)
                    nc.vector.tensor_scalar(asb, pnat, rcol, None, OP.mult)
                    nc.sync.dma_start(attn[h, i * P:(i + 1) * P, :], asb)
                    pump()

                # queue PV of the previous head; this head's PV queues next
                if prev_pv is not None:
                    filler.extend(pv_closures(*prev_pv))
                prev_pv = (h, PT)
            qkt_cur = qkt_next

        for c in pv_closures(*prev_pv):
            c()
        while filler:
            pump()

    # ---------------- Stage D: FC + residual ----------------
    with ExitStack() as stD:
        pd = stD.enter_context(tc.tile_pool(name="pd", bufs=2))
        pdb = stD.enter_context(tc.tile_pool(name="pdb", bufs=1))
        psD = stD.enter_context(tc.tile_pool(name="psD", bufs=3, space="PSUM"))

        bfc_bcast = pdb.tile([P, C], F32)
        bfc_ap = bass.AP(tensor=b_fc.ap().tensor, offset=0, ap=[[0, P], [1, C]])
        nc.gpsimd.dma_start(out=bfc_bcast, in_=bfc_ap)

        for i in range(TT):
            x_re = pd.tile([P, C], F32, tag="xre")
            nc.sync.dma_start(x_re, x[i * P:(i + 1) * P, :])
            # fold b_fc into the residual in place
            nc.vector.tensor_tensor(x_re, x_re, bfc_bcast, OP.add)
            osb = pd.tile([P, C], F32, tag="osb")
            for half in range(2):
                ps_fc = psD.tile([P, 512], F32, tag="fc")
                for j in range(KO):
                    nc.tensor.matmul(ps_fc, oT[:, j, i * P:(i + 1) * P],
                                     wf_r[:, j, half * 512:(half + 1) * 512],
                                     start=(j == 0), stop=(j == KO - 1))
                nc.vector.tensor_tensor(osb[:, half * 512:(half + 1) * 512],
                                        ps_fc,
                                        x_re[:, half * 512:(half + 1) * 512],
                                        OP.add)
            nc.sync.dma_start(out[i * P:(i + 1) * P, :], osb)

    singles_cm.__exit__(None, None, None)


_NC_CACHE = None


def kernel(x, ln_g, ln_b, W_qkv, b_qkv, W_fc, b_fc):
    global _NC_CACHE
    if _NC_CACHE is None:
        _NC_CACHE = build_nc()
    nc = _NC_CACHE

    x = np.ascontiguousarray(np.asarray(x, dtype=np.float32))
    shared = {
        "ln_g": np.ascontiguousarray(np.asarray(ln_g, np.float32)),
        "ln_b": np.ascontiguousarray(np.asarray(ln_b, np.float32)),
        "w_qkv": np.ascontiguousarray(np.asarray(W_qkv, np.float32)),
        "b_qkv": np.ascontiguousarray(np.asarray(b_qkv, np.float32)),
        "w_fc": np.ascontiguousarray(np.asarray(W_fc, np.float32)),
        "b_fc": np.ascontiguousarray(np.asarray(b_fc, np.float32)),
    }
    in_maps = [{"x": x[b], **shared} for b in range(B)]
    res = run_bass_kernel_spmd(nc, in_maps, core_ids=list(range(N_CORES)))
    out = np.stack([res.results[b]["out"] for b in range(B)])
    attn = np.stack([res.results[b]["attn"] for b in range(B)])
    return out, attn
